# revision 51
# baseline (speedup 1.0000x reference)
"""Causal self-attention (B=2, S=2048, D=1024, H=16) on 8 TRN2 NeuronCores.

Sharding: core c -> batch b = c//4, head group g = c%4 (4 heads each).
Each core computes the qkv projection for its heads, RoPE, causal flash
attention, and a partial out-projection (row-parallel); the host sums the
4 partials per batch.

Layout strategy (everything "transposed", seq on the free axis):
  xt  = x[b]^T                  [D, S]   bf16 (host-prepped)
  Qt/Kt[m, s] per head          computed as  W[:,m]^T @ xt  (lhsT = W slice)
  V natural [s, m]              computed as  xt_tile^T @ Wv
  St[k, q]  = Kt_tile^T @ Qt    -> exp -> causal mask -> Pt (bf16)
  Ot'[m+1, q] = [V|1]^T @ Pt    (row m==HD is the softmax denominator l[q])
  y^T[n, s] = Wo[:,n]^T @ (Ot/l)  accumulated over m tiles; host sums cores.

RoPE: interleaved rotate made partition-aligned by permuting W columns on
the host; on-device combine = 2 muls + stream_shuffle + add (Vector).

Perf structure (v3, 168us vs 185us baseline):
 - All inputs land via 9 large DMAs on the sync HWDGE ring instead of ~70
   small ones (each dma_start costs ~615ns of serialized issue), so the
   first matmul starts ~3us in and the HAM clock-gate (which halves the PE
   clock after ~3.4us of low activity) never re-throttles mid-kernel.
 - 10 dummy matmuls at t=0 warm the PE clock while the DMAs stream; 8 more
   before the last out-projection cover the exp-bound tail.
 - Work is emitted chunk-by-chunk with attention of chunk ci interleaved
   with the qkv of chunk ci+2 and the out-projection of ci-1 (PE filler
   for the ACT-exp-paced softmax, ~1us per key-tile pair). Emission order
   per engine follows dataflow order pair-by-pair: reordering across pairs
   creates multi-us cross-FIFO convoys (engines execute in FIFO order).
 - Output is evicted to bf16 and written with 2 large DMAs per chunk.
 - RoPE add on GpSimd; l broadcast on GpSimd; evictions on Vector (last
   chunk on Scalar, which is idle once exp finishes).
"""

from contextlib import ExitStack

import numpy as np
import ml_dtypes

import concourse.bass as bass
import concourse.tile as tile
import concourse.mybir as mybir
from concourse import bacc
from concourse.bass_utils import run_bass_kernel_spmd

HD = 64          # head dim
CH = 512         # seq chunk (one PSUM bank of fp32)
_SHUF = [(i + 16) % 32 for i in range(32)]  # swap 16-halves in each quadrant


def rope_perm():
    """Within-head output-column permutation: local row r <- reference col."""
    perm = np.zeros(HD, dtype=np.int64)
    for r in range(HD):
        q, pos = divmod(r, 32)
        x2 = pos >= 16
        f = q * 16 + (pos % 16)
        perm[r] = 2 * f + (1 if x2 else 0)
    return perm


def rope_tables(rope_cos, rope_sin, S):
    """cos/sin tables [128, S] fp32 aligned with the permuted Qt/Kt rows."""
    cs = np.zeros((128, S), np.float32)
    sn = np.zeros((128, S), np.float32)
    for r in range(128):
        rr = r % HD
        q, pos = divmod(rr, 32)
        x2 = pos >= 16
        f = q * 16 + (pos % 16)
        cs[r] = rope_cos[:S, f]
        sn[r] = rope_sin[:S, f] * (-1.0 if x2 else 1.0)
    return cs, sn


def merge_prop(a, b):
    """Merge two unit lists, advancing each proportionally to its length."""
    out = []
    ia = ib = 0
    while ia < len(a) or ib < len(b):
        fa = ia / len(a) if a else 1.0
        fb = ib / len(b) if b else 1.0
        if ia < len(a) and (ib >= len(b) or fa <= fb):
            out.append(a[ia]); ia += 1
        else:
            out.append(b[ib]); ib += 1
    return out


def build_core(nc, S, D, HC):
    """Emit the per-core kernel IR. HC = heads on this core."""
    DT = D // 128           # contraction tiles over model dim
    M = HC * HD             # local qkv width
    MT = M // 128           # m tiles
    NCH = S // CH           # seq chunks
    KPC = CH // 128         # key tiles per chunk
    NT = D // 128           # out-proj n tiles
    NT2 = NT // 2
    HP = HC // 2            # head pairs
    fp32, bf16 = mybir.dt.float32, mybir.dt.bfloat16
    SCALE = float(HD) ** -0.5

    xt_d = nc.declare_dram_parameter("xt", [NCH, 128, DT * CH], bf16, isOutput=False)
    wq_d = nc.declare_dram_parameter("wq", [128, DT * M], bf16, isOutput=False)
    wk_d = nc.declare_dram_parameter("wk", [128, DT * M], bf16, isOutput=False)
    wv_d = nc.declare_dram_parameter("wv", [128, DT * M], bf16, isOutput=False)
    wo_d = nc.declare_dram_parameter("wo", [128, MT * D], bf16, isOutput=False)
    csn_d = nc.declare_dram_parameter("csn", [128, 2 * NCH * CH], bf16, isOutput=False)
    yt_d = nc.declare_dram_parameter("yt", [NCH, 2, 128, NT2 * CH], bf16, isOutput=True)

    with tile.TileContext(nc) as tc, ExitStack() as ctx:
        persist = ctx.enter_context(tc.tile_pool(name="persist", bufs=1))
        mm_ps = ctx.enter_context(tc.tile_pool(name="mm_ps", bufs=2, space="PSUM"))
        st_ps = ctx.enter_context(tc.tile_pool(name="st_ps", bufs=2, space="PSUM"))
        ot_ps = ctx.enter_context(tc.tile_pool(name="ot_ps", bufs=2, space="PSUM"))
        work = ctx.enter_context(tc.tile_pool(name="work", bufs=3))
        pt_pool = ctx.enter_context(tc.tile_pool(name="ptp", bufs=20))
        out_pool = ctx.enter_context(tc.tile_pool(name="outp", bufs=2))

        # ---- persistent tiles -------------------------------------------
        xt = [persist.tile([128, DT, CH], bf16, name=f"xt_{c}") for c in range(NCH)]
        wq = persist.tile([128, DT, M], bf16)
        wk = persist.tile([128, DT, M], bf16)
        wv = persist.tile([128, DT, M], bf16)
        wo = persist.tile([128, MT, D], bf16)
        csn = persist.tile([128, 2, NCH, CH], bf16)
        qt = [persist.tile([128, MT, CH], bf16, name=f"qt_{c}") for c in range(NCH)]
        kt = [persist.tile([128, MT, CH], bf16, name=f"kt_{c}") for c in range(NCH)]
        vsb = [persist.tile([128, KPC, HC, HD + 1], bf16, name=f"vsb_{c}")
               for c in range(NCH)]
        otn = [persist.tile([128, MT, CH], bf16, name=f"otn_{c}") for c in range(NCH)]
        cmask = persist.tile([128, 2, 128], bf16)

        # ---- input DMAs: few and large, on the sync HWDGE ring.  The
        # chunk-0 slice of the rope tables rides ahead of the full table so
        # the first rope (chunk 0 -> the first exp) isn't gated on 1MB.
        csn_dv = csn_d.rearrange("p (i c s) -> p i c s", i=2, c=NCH)
        nc.sync.dma_start(out=wq.rearrange("p t m -> p (t m)"), in_=wq_d[:, :])
        nc.sync.dma_start(out=csn[:, :, 0, :], in_=csn_dv[:, :, 0, :])
        nc.sync.dma_start(
            out=xt[0].rearrange("p t s -> p (t s)"), in_=xt_d[0])
        nc.sync.dma_start(out=csn[:, :, 1:, :], in_=csn_dv[:, :, 1:, :])
        nc.sync.dma_start(out=wk.rearrange("p t m -> p (t m)"), in_=wk_d[:, :])
        nc.sync.dma_start(out=wv.rearrange("p t m -> p (t m)"), in_=wv_d[:, :])
        for c in range(1, NCH):
            nc.sync.dma_start(
                out=xt[c].rearrange("p t s -> p (t s)"), in_=xt_d[c])
        nc.sync.dma_start(out=wo.rearrange("p t n -> p (t n)"), in_=wo_d[:, :])

        # ---- PE warmup: HAM clock un-throttle while DMAs stream ---------
        # ~128 small matmuls keep the PE busy (and at full clock) until the
        # first real matmul's inputs land (~16us); a dummy exp preloads the
        # ACT table set (~2.7us) off the first real exp's critical path.
        warm_w = persist.tile([128, 128], bf16)
        warm_x = persist.tile([128, CH], bf16)
        nc.vector.memset(warm_w[:], 0.0)
        nc.vector.memset(warm_x[:], 0.0)
        warm_act = work.tile([128, 16], fp32, tag="wact")
        nc.scalar.activation(out=warm_act[:], in_=warm_w[:, 0:16],
                             func=mybir.ActivationFunctionType.Exp)
        warm_ps = mm_ps.tile([128, CH], fp32, tag="mm", name="mmps")
        for _ in range(128):
            nc.tensor.matmul(warm_ps[:, 0:128], warm_w[:], warm_w[:],
                             start=True, stop=True)

        # causal mask for the 128-col diagonal window (both head slots):
        # keep j - p >= 0 (query-local j, key-local p) - identical for
        # every diagonal tile; columns past the window are never masked.
        # Built on GpSimd while it is otherwise idle.
        nc.gpsimd.memset(cmask[:], 1.0)
        for i in range(2):
            nc.gpsimd.affine_select(
                out=cmask[:, i, :], in_=cmask[:, i, :],
                compare_op=mybir.AluOpType.is_ge, fill=0.0,
                base=0, pattern=[[1, 128]], channel_multiplier=-1,
            )
        for c in range(NCH):
            nc.vector.memset(vsb[c][:, :, :, HD:HD + 1], 1.0)

        # ---- emission units ---------------------------------------------
        def qk_part(ci, wt, dst):
            for mt in range(MT):
                ps = mm_ps.tile([128, CH], fp32, tag="mm", name="mmps")
                for dt in range(DT):
                    nc.tensor.matmul(
                        ps[:],
                        wt[:, dt, mt * 128:(mt + 1) * 128],
                        xt[ci][:, dt, :],
                        start=(dt == 0), stop=(dt == DT - 1),
                    )
                p1 = work.tile([128, CH], fp32, tag="p1")
                p2 = work.tile([128, CH], fp32, tag="p2")
                p2s = work.tile([128, CH], fp32, tag="p2s")
                nc.vector.tensor_mul(p1[:], ps[:], csn[:, 0, ci, :])
                nc.vector.tensor_mul(p2[:], ps[:], csn[:, 1, ci, :])
                nc.vector.stream_shuffle(p2s[:], p2[:], mask=_SHUF)
                nc.vector.tensor_add(dst[ci][:, mt, :], p1[:], p2s[:])

        def v_part(ci):
            for sl in range(KPC):
                ps = mm_ps.tile([128, M], fp32, tag="mm", name="vps")
                for dt in range(DT):
                    nc.tensor.matmul(
                        ps[:],
                        xt[ci][:, dt, sl * 128:(sl + 1) * 128],
                        wv[:, dt, :],
                        start=(dt == 0), stop=(dt == DT - 1),
                    )
                nc.vector.tensor_copy(
                    vsb[ci][:, sl, :, 0:HD],
                    ps.rearrange("p (h d) -> p h d", h=HC),
                )

        def attn_part(ci, hp):
            nkt = (ci + 1) * KPC
            heads = (2 * hp, 2 * hp + 1)
            mt = hp
            ots = {}
            for h in heads:
                ots[h] = ot_ps.tile([128, CH], fp32, tag="ot", name=f"ot_{h}")
            pts = []

            def av_kj(kj):
                pt, trim = pts[kj]
                kc, kl = divmod(kj, KPC)
                for i, h in enumerate(heads):
                    nc.tensor.matmul(
                        ots[h][0:HD + 1, trim:],
                        vsb[kc][:, kl, h, :],
                        pt[:, i, trim:],
                        start=(kj == 0), stop=(kj == nkt - 1),
                    )

            for kj in range(nkt):
                tidx = kj - ci * KPC
                trim = max(0, tidx) * 128
                kc, kl = divmod(kj, KPC)
                stp = st_ps.tile([128, 2, CH], fp32, tag="st")
                for i, h in enumerate(heads):
                    base = (h % 2) * 64
                    nc.tensor.matmul(
                        stp[:, i, trim:],
                        kt[kc][base:base + HD, mt, kl * 128:(kl + 1) * 128],
                        qt[ci][base:base + HD, mt, trim:],
                        start=True, stop=True,
                    )
                pt = pt_pool.tile([128, 2, CH], bf16, tag="pt")
                nc.scalar.activation(
                    out=pt[:, :, trim:], in_=stp[:, :, trim:],
                    func=mybir.ActivationFunctionType.Exp, scale=SCALE,
                )
                if tidx >= 0:
                    nc.vector.tensor_mul(
                        pt[:, :, trim:trim + 128], pt[:, :, trim:trim + 128],
                        cmask[:, :, :],
                    )
                pts.append((pt, trim))
                # AV for kj-2 rides inside the exp-paced loop: QK(kj) can
                # only issue once exp(kj-2) freed its st bank (2 bufs), so
                # this adds zero stalls and empties the post-loop PE block
                # that otherwise delays the next pair's exps.
                if kj >= 2:
                    av_kj(kj - 2)
            av_kj(nkt - 2)
            av_kj(nkt - 1)
            for h in heads:
                base = (h % 2) * 64
                ot = ots[h]
                l_sb = work.tile([1, CH], fp32, tag="l")
                nc.vector.tensor_copy(l_sb[:], ot[HD:HD + 1, :])
                rl = work.tile([1, CH], fp32, tag="rl")
                nc.vector.reciprocal_approx_fast(rl[:], l_sb[:])
                lb = work.tile([64, CH], fp32, tag="lb")
                nc.gpsimd.partition_broadcast(lb[:], rl[0:1, :])
                nc.vector.tensor_mul(
                    otn[ci][base:base + HD, mt, :], ot[0:HD, :], lb[:],
                )

        yts = {}

        def proj_part(ci, half):
            if half == 0:
                yts[ci] = out_pool.tile([128, NT, CH], bf16, tag="yt", name="yt")
            for nt in range(half * NT2, (half + 1) * NT2):
                ps = mm_ps.tile([128, CH], fp32, tag="mm", name="mmps")
                for mt2 in range(MT):
                    nc.tensor.matmul(
                        ps[:],
                        wo[:, mt2, nt * 128:(nt + 1) * 128],
                        otn[ci][:, mt2, :],
                        start=(mt2 == 0), stop=(mt2 == MT - 1),
                    )
                # last chunk: alternate Scalar/Vector (both idle once the
                # exps finish) so the tail eviction chain halves.
                if ci == NCH - 1 and nt % 2 == 0:
                    nc.scalar.copy(yts[ci][:, nt, :], ps[:])
                else:
                    nc.vector.tensor_copy(yts[ci][:, nt, :], ps[:])
            nc.sync.dma_start(
                out=yt_d[ci, half],
                in_=yts[ci][:, half * NT2:(half + 1) * NT2, :]
                .rearrange("p t s -> p (t s)"),
            )

        from itertools import zip_longest

        def interleave(*streams):
            for group in zip_longest(*streams):
                for fn in group:
                    if fn is not None:
                        fn()

        def qkv_units(ci):
            return [lambda: qk_part(ci, wq, qt),
                    lambda: qk_part(ci, wk, kt),
                    lambda: v_part(ci)]

        def attn_units(ci):
            return [(lambda hp=hp: attn_part(ci, hp)) for hp in range(HP)]

        def proj_units(ci):
            return [lambda: proj_part(ci, 0), lambda: proj_part(ci, 1)]

        interleave(qkv_units(0))
        if NCH == 1:
            interleave(attn_units(0))
            interleave(proj_units(0))
        else:
            interleave(qkv_units(1))
            for ci in range(NCH - 2):
                streams = [attn_units(ci), qkv_units(ci + 2)]
                if ci >= 1:
                    streams.append(proj_units(ci - 1))
                interleave(*streams)
            interleave(attn_units(NCH - 2),
                       proj_units(NCH - 3) if NCH >= 3 else [])
            interleave(attn_units(NCH - 1),
                       proj_units(NCH - 2) if NCH >= 2 else [])
            ka_ps = mm_ps.tile([128, CH], fp32, tag="mm", name="ka_ps")
            for _ in range(8):
                nc.tensor.matmul(ka_ps[:], warm_w[:],
                                 otn[NCH - 1][:, 0, :], start=True, stop=True)
            interleave(proj_units(NCH - 1))




_CACHE = {}


def _get_nc(S, D, HC):
    key = (S, D, HC)
    if key not in _CACHE:
        nc = bacc.Bacc(None, target_bir_lowering=False)
        build_core(nc, S, D, HC)
        nc.compile()
        _CACHE[key] = nc
    return _CACHE[key]


def make_in_maps(x, rope_cos, rope_sin, W_qkv, W_out, n_cores=8):
    B, S, D = x.shape
    H = 16
    groups = n_cores // B          # head groups per batch
    HC = H // groups               # heads per core
    M = HC * HD
    MT = M // 128
    DT, NCH = D // 128, S // CH
    perm = rope_perm()
    bf16 = ml_dtypes.bfloat16
    cs, sn = rope_tables(np.asarray(rope_cos), np.asarray(rope_sin), S)
    csn = np.stack([cs.reshape(128, NCH * CH), sn.reshape(128, NCH * CH)],
                   axis=1).reshape(128, 2 * NCH * CH)
    in_maps = []
    xtb_cache = {}
    for c in range(n_cores):
        b, g = divmod(c, groups)
        heads = np.arange(g * HC, (g + 1) * HC)
        qcols = np.concatenate([h * HD + perm for h in heads])
        vcols = np.concatenate([2 * D + h * HD + np.arange(HD) for h in heads])
        if b not in xtb_cache:
            xtb_cache[b] = np.ascontiguousarray(
                np.asarray(x[b]).T.reshape(DT, 128, NCH, CH)
                .transpose(2, 1, 0, 3).reshape(NCH, 128, DT * CH)
            ).astype(bf16)

        def wfmt(wcols):
            return np.ascontiguousarray(
                wcols.reshape(DT, 128, M).transpose(1, 0, 2).reshape(128, DT * M)
            ).astype(bf16)

        wo_np = np.ascontiguousarray(
            W_out[g * M:(g + 1) * M, :].reshape(MT, 128, D)
            .transpose(1, 0, 2).reshape(128, MT * D)).astype(bf16)
        in_maps.append({
            "xt": xtb_cache[b],
            "wq": wfmt(W_qkv[:, qcols]),
            "wk": wfmt(W_qkv[:, D + qcols]),
            "wv": wfmt(W_qkv[:, vcols]),
            "wo": wo_np,
            "csn": np.ascontiguousarray(csn).astype(bf16),
        })
    return in_maps


def unshard_out(res, B, S, D, n_cores=8):
    NCH, NT = S // CH, D // 128
    NT2 = NT // 2
    out = np.zeros((B, S, D), np.float32)
    for c in range(n_cores):
        yt = res.results[c]["yt"].astype(np.float32)  # [NCH, 2, 128, NT2*CH]
        ytf = (yt.reshape(NCH, 2, 128, NT2, CH)
               .transpose(1, 3, 2, 0, 4).reshape(D, S))
        out[c // (n_cores // B)] += ytf.T
    return out


def kernel(x, rope_cos, rope_sin, W_qkv, W_out):
    x = np.asarray(x)
    W_qkv = np.asarray(W_qkv)
    W_out = np.asarray(W_out)
    B, S, D = x.shape
    n_cores = 8
    HC = 16 // (n_cores // B)
    in_maps = make_in_maps(x, rope_cos, rope_sin, W_qkv, W_out, n_cores)
    nc = _get_nc(S, D, HC)
    res = run_bass_kernel_spmd(nc, in_maps, list(range(n_cores)))
    return unshard_out(res, B, S, D, n_cores)



# revision 52
# speedup vs baseline: 1.2230x; 1.2230x over previous
"""Causal self-attention (B=2, S=2048, D=1024, H=16) on 8 TRN2 NeuronCores.

Sharding: core c -> batch b = c//4, head group g = c%4 (4 heads each).
Each core computes the qkv projection for its heads, RoPE, causal flash
attention, and a partial out-projection (row-parallel); the host sums the
4 partials per batch.

Layout strategy (everything "transposed", seq on the free axis):
  xt  = x[b]^T                  [D, S]   bf16 (host-prepped)
  Qt/Kt[m, s] per head          computed as  W[:,m]^T @ xt  (lhsT = W slice)
  V natural [s, m]              computed as  xt_tile^T @ Wv
  St[k, q]  = Kt_tile^T @ Qt    -> exp -> causal mask -> Pt (bf16)
  Ot'[m+1, q] = [V|1]^T @ Pt    (row m==HD is the softmax denominator l[q])
  y^T[n, s] = Wo[:,n]^T @ (Ot/l)  accumulated over m tiles; host sums cores.

RoPE: interleaved rotate made partition-aligned by permuting W columns on
the host; on-device combine = 2 muls + stream_shuffle + add (Vector).

Perf structure (v3, 168us vs 185us baseline):
 - All inputs land via 9 large DMAs on the sync HWDGE ring instead of ~70
   small ones (each dma_start costs ~615ns of serialized issue), so the
   first matmul starts ~3us in and the HAM clock-gate (which halves the PE
   clock after ~3.4us of low activity) never re-throttles mid-kernel.
 - 10 dummy matmuls at t=0 warm the PE clock while the DMAs stream; 8 more
   before the last out-projection cover the exp-bound tail.
 - Work is emitted chunk-by-chunk with attention of chunk ci interleaved
   with the qkv of chunk ci+2 and the out-projection of ci-1 (PE filler
   for the ACT-exp-paced softmax, ~1us per key-tile pair). Emission order
   per engine follows dataflow order pair-by-pair: reordering across pairs
   creates multi-us cross-FIFO convoys (engines execute in FIFO order).
 - Output is evicted to bf16 and written with 2 large DMAs per chunk.
 - RoPE add on GpSimd; l broadcast on GpSimd; evictions on Vector (last
   chunk on Scalar, which is idle once exp finishes).
"""

from contextlib import ExitStack

import numpy as np
import ml_dtypes

import concourse.bass as bass
import concourse.tile as tile
import concourse.mybir as mybir
from concourse import bacc
from concourse.bass_utils import run_bass_kernel_spmd

HD = 64          # head dim
CH = 512         # seq chunk (one PSUM bank of fp32)
_SHUF = [(i + 16) % 32 for i in range(32)]  # swap 16-halves in each quadrant


def rope_perm():
    """Within-head output-column permutation: local row r <- reference col."""
    perm = np.zeros(HD, dtype=np.int64)
    for r in range(HD):
        q, pos = divmod(r, 32)
        x2 = pos >= 16
        f = q * 16 + (pos % 16)
        perm[r] = 2 * f + (1 if x2 else 0)
    return perm


def rope_tables(rope_cos, rope_sin, S):
    """cos/sin tables [128, S] fp32 aligned with the permuted Qt/Kt rows."""
    cs = np.zeros((128, S), np.float32)
    sn = np.zeros((128, S), np.float32)
    for r in range(128):
        rr = r % HD
        q, pos = divmod(rr, 32)
        x2 = pos >= 16
        f = q * 16 + (pos % 16)
        cs[r] = rope_cos[:S, f]
        sn[r] = rope_sin[:S, f] * (-1.0 if x2 else 1.0)
    return cs, sn


def merge_prop(a, b):
    """Merge two unit lists, advancing each proportionally to its length."""
    out = []
    ia = ib = 0
    while ia < len(a) or ib < len(b):
        fa = ia / len(a) if a else 1.0
        fb = ib / len(b) if b else 1.0
        if ia < len(a) and (ib >= len(b) or fa <= fb):
            out.append(a[ia]); ia += 1
        else:
            out.append(b[ib]); ib += 1
    return out


def build_core(nc, S, D, HC):
    """Emit the per-core kernel IR. HC = heads on this core."""
    DT = D // 128           # contraction tiles over model dim
    M = HC * HD             # local qkv width
    MT = M // 128           # m tiles
    NCH = S // CH           # seq chunks
    KPC = CH // 128         # key tiles per chunk
    NT = D // 128           # out-proj n tiles
    NT2 = NT // 2
    HP = HC // 2            # head pairs
    fp32, bf16 = mybir.dt.float32, mybir.dt.bfloat16
    SCALE = float(HD) ** -0.5

    xt_d = nc.declare_dram_parameter("xt", [NCH, 128, DT * CH], bf16, isOutput=False)
    wq_d = nc.declare_dram_parameter("wq", [128, DT * M], bf16, isOutput=False)
    wk_d = nc.declare_dram_parameter("wk", [128, DT * M], bf16, isOutput=False)
    wv_d = nc.declare_dram_parameter("wv", [128, DT * M], bf16, isOutput=False)
    wo_d = nc.declare_dram_parameter("wo", [128, MT * D], bf16, isOutput=False)
    csn_d = nc.declare_dram_parameter("csn", [128, 2 * NCH * CH], bf16, isOutput=False)
    yt_d = nc.declare_dram_parameter("yt", [NCH, 2, 128, NT2 * CH], bf16, isOutput=True)

    with tile.TileContext(nc) as tc, ExitStack() as ctx:
        persist = ctx.enter_context(tc.tile_pool(name="persist", bufs=1))
        mm_ps = ctx.enter_context(tc.tile_pool(name="mm_ps", bufs=2, space="PSUM"))
        st_ps = ctx.enter_context(tc.tile_pool(name="st_ps", bufs=2, space="PSUM"))
        ot_ps = ctx.enter_context(tc.tile_pool(name="ot_ps", bufs=2, space="PSUM"))
        work = ctx.enter_context(tc.tile_pool(name="work", bufs=3))
        pt_pool = ctx.enter_context(tc.tile_pool(name="ptp", bufs=20))
        out_pool = ctx.enter_context(tc.tile_pool(name="outp", bufs=2))

        # ---- persistent tiles -------------------------------------------
        xt = [persist.tile([128, DT, CH], bf16, name=f"xt_{c}") for c in range(NCH)]
        wq = persist.tile([128, DT, M], bf16)
        wk = persist.tile([128, DT, M], bf16)
        wv = persist.tile([128, DT, M], bf16)
        wo = persist.tile([128, MT, D], bf16)
        csn = persist.tile([128, 2, NCH, CH], bf16)
        qt = [persist.tile([128, MT, CH], bf16, name=f"qt_{c}") for c in range(NCH)]
        kt = [persist.tile([128, MT, CH], bf16, name=f"kt_{c}") for c in range(NCH)]
        vsb = [persist.tile([128, KPC, HC, HD + 1], bf16, name=f"vsb_{c}")
               for c in range(NCH)]
        otn = [persist.tile([128, MT, CH], bf16, name=f"otn_{c}") for c in range(NCH)]
        cmask = persist.tile([128, 2, 128], bf16)

        # ---- input DMAs: few and large, on the sync HWDGE ring.  The
        # chunk-0 slice of the rope tables rides ahead of the full table so
        # the first rope (chunk 0 -> the first exp) isn't gated on 1MB.
        csn_dv = csn_d.rearrange("p (i c s) -> p i c s", i=2, c=NCH)
        nc.sync.dma_start(out=wq.rearrange("p t m -> p (t m)"), in_=wq_d[:, :])
        nc.sync.dma_start(out=csn[:, :, 0, :], in_=csn_dv[:, :, 0, :])
        nc.sync.dma_start(
            out=xt[0].rearrange("p t s -> p (t s)"), in_=xt_d[0])
        nc.sync.dma_start(out=csn[:, :, 1:, :], in_=csn_dv[:, :, 1:, :])
        nc.sync.dma_start(out=wk.rearrange("p t m -> p (t m)"), in_=wk_d[:, :])
        nc.sync.dma_start(out=wv.rearrange("p t m -> p (t m)"), in_=wv_d[:, :])
        for c in range(1, NCH):
            nc.sync.dma_start(
                out=xt[c].rearrange("p t s -> p (t s)"), in_=xt_d[c])
        nc.sync.dma_start(out=wo.rearrange("p t n -> p (t n)"), in_=wo_d[:, :])

        # ---- PE warmup: HAM clock un-throttle while DMAs stream ---------
        # ~128 small matmuls keep the PE busy (and at full clock) until the
        # first real matmul's inputs land (~16us); a dummy exp preloads the
        # ACT table set (~2.7us) off the first real exp's critical path.
        warm_w = persist.tile([128, 128], bf16)
        warm_x = persist.tile([128, CH], bf16)
        nc.vector.memset(warm_w[:], 0.0)
        nc.vector.memset(warm_x[:], 0.0)
        warm_act = work.tile([128, 16], fp32, tag="wact")
        nc.scalar.activation(out=warm_act[:], in_=warm_w[:, 0:16],
                             func=mybir.ActivationFunctionType.Exp)
        warm_ps = mm_ps.tile([128, CH], fp32, tag="mm", name="mmps")
        for _ in range(128):
            nc.tensor.matmul(warm_ps[:, 0:128], warm_w[:], warm_w[:],
                             start=True, stop=True)

        # causal mask for the 128-col diagonal window (both head slots):
        # keep j - p >= 0 (query-local j, key-local p) - identical for
        # every diagonal tile; columns past the window are never masked.
        # Built on GpSimd while it is otherwise idle.
        nc.gpsimd.memset(cmask[:], 1.0)
        for i in range(2):
            nc.gpsimd.affine_select(
                out=cmask[:, i, :], in_=cmask[:, i, :],
                compare_op=mybir.AluOpType.is_ge, fill=0.0,
                base=0, pattern=[[1, 128]], channel_multiplier=-1,
            )
        for c in range(NCH):
            nc.vector.memset(vsb[c][:, :, :, HD:HD + 1], 1.0)

        # ---- emission units ---------------------------------------------
        def qk_part(ci, wt, dst):
            for mt in range(MT):
                ps = mm_ps.tile([128, CH], fp32, tag="mm", name="mmps")
                for dt in range(DT):
                    nc.tensor.matmul(
                        ps[:],
                        wt[:, dt, mt * 128:(mt + 1) * 128],
                        xt[ci][:, dt, :],
                        start=(dt == 0), stop=(dt == DT - 1),
                    )
                p1 = work.tile([128, CH], fp32, tag="p1")
                p2 = work.tile([128, CH], fp32, tag="p2")
                p2s = work.tile([128, CH], fp32, tag="p2s")
                nc.vector.tensor_mul(p1[:], ps[:], csn[:, 0, ci, :])
                nc.vector.tensor_mul(p2[:], ps[:], csn[:, 1, ci, :])
                nc.vector.stream_shuffle(p2s[:], p2[:], mask=_SHUF)
                nc.vector.tensor_add(dst[ci][:, mt, :], p1[:], p2s[:])

        def v_part(ci):
            for sl in range(KPC):
                ps = mm_ps.tile([128, M], fp32, tag="mm", name="vps")
                for dt in range(DT):
                    nc.tensor.matmul(
                        ps[:],
                        xt[ci][:, dt, sl * 128:(sl + 1) * 128],
                        wv[:, dt, :],
                        start=(dt == 0), stop=(dt == DT - 1),
                    )
                nc.vector.tensor_copy(
                    vsb[ci][:, sl, :, 0:HD],
                    ps.rearrange("p (h d) -> p h d", h=HC),
                )

        def attn_part(ci, hp):
            nkt = (ci + 1) * KPC
            heads = (2 * hp, 2 * hp + 1)
            mt = hp
            ots = {}
            for h in heads:
                ots[h] = ot_ps.tile([128, CH], fp32, tag="ot", name=f"ot_{h}")
            pts = []
            for kj in range(nkt):
                tidx = kj - ci * KPC
                trim = max(0, tidx) * 128
                kc, kl = divmod(kj, KPC)
                stp = st_ps.tile([128, 2, CH], fp32, tag="st")
                for i, h in enumerate(heads):
                    base = (h % 2) * 64
                    nc.tensor.matmul(
                        stp[:, i, trim:],
                        kt[kc][base:base + HD, mt, kl * 128:(kl + 1) * 128],
                        qt[ci][base:base + HD, mt, trim:],
                        start=True, stop=True,
                    )
                pt = pt_pool.tile([128, 2, CH], bf16, tag="pt")
                nc.scalar.activation(
                    out=pt[:, :, trim:], in_=stp[:, :, trim:],
                    func=mybir.ActivationFunctionType.Exp, scale=SCALE,
                )
                if tidx >= 0:
                    nc.vector.tensor_mul(
                        pt[:, :, trim:trim + 128], pt[:, :, trim:trim + 128],
                        cmask[:, :, :],
                    )
                pts.append((pt, trim))
            for i, h in enumerate(heads):
                for kj in range(nkt):
                    pt, trim = pts[kj]
                    kc, kl = divmod(kj, KPC)
                    nc.tensor.matmul(
                        ots[h][0:HD + 1, trim:],
                        vsb[kc][:, kl, h, :],
                        pt[:, i, trim:],
                        start=(kj == 0), stop=(kj == nkt - 1),
                    )
            for h in heads:
                base = (h % 2) * 64
                ot = ots[h]
                l_sb = work.tile([1, CH], fp32, tag="l")
                nc.vector.tensor_copy(l_sb[:], ot[HD:HD + 1, :])
                rl = work.tile([1, CH], fp32, tag="rl")
                nc.vector.reciprocal_approx_fast(rl[:], l_sb[:])
                lb = work.tile([64, CH], fp32, tag="lb")
                nc.gpsimd.partition_broadcast(lb[:], rl[0:1, :])
                nc.vector.tensor_mul(
                    otn[ci][base:base + HD, mt, :], ot[0:HD, :], lb[:],
                )

        yts = {}

        def proj_part(ci, half):
            if half == 0:
                yts[ci] = out_pool.tile([128, NT, CH], bf16, tag="yt", name="yt")
            for nt in range(half * NT2, (half + 1) * NT2):
                ps = mm_ps.tile([128, CH], fp32, tag="mm", name="mmps")
                for mt2 in range(MT):
                    nc.tensor.matmul(
                        ps[:],
                        wo[:, mt2, nt * 128:(nt + 1) * 128],
                        otn[ci][:, mt2, :],
                        start=(mt2 == 0), stop=(mt2 == MT - 1),
                    )
                # last chunk: alternate Scalar/Vector (both idle once the
                # exps finish) so the tail eviction chain halves.
                if ci == NCH - 1 and nt % 2 == 0:
                    nc.scalar.copy(yts[ci][:, nt, :], ps[:])
                else:
                    nc.vector.tensor_copy(yts[ci][:, nt, :], ps[:])
            nc.sync.dma_start(
                out=yt_d[ci, half],
                in_=yts[ci][:, half * NT2:(half + 1) * NT2, :]
                .rearrange("p t s -> p (t s)"),
            )

        from itertools import zip_longest

        def interleave(*streams):
            for group in zip_longest(*streams):
                for fn in group:
                    if fn is not None:
                        fn()

        def qkv_units(ci):
            return [lambda: qk_part(ci, wq, qt),
                    lambda: qk_part(ci, wk, kt),
                    lambda: v_part(ci)]

        def attn_units(ci):
            return [(lambda hp=hp: attn_part(ci, hp)) for hp in range(HP)]

        def proj_units(ci):
            return [lambda: proj_part(ci, 0), lambda: proj_part(ci, 1)]

        interleave(qkv_units(0))
        if NCH == 1:
            interleave(attn_units(0))
            interleave(proj_units(0))
        else:
            interleave(qkv_units(1))
            for ci in range(NCH - 2):
                streams = [attn_units(ci), qkv_units(ci + 2)]
                if ci >= 1:
                    streams.append(proj_units(ci - 1))
                interleave(*streams)
            interleave(attn_units(NCH - 2),
                       proj_units(NCH - 3) if NCH >= 3 else [])
            interleave(attn_units(NCH - 1),
                       proj_units(NCH - 2) if NCH >= 2 else [])
            ka_ps = mm_ps.tile([128, CH], fp32, tag="mm", name="ka_ps")
            for _ in range(8):
                nc.tensor.matmul(ka_ps[:], warm_w[:],
                                 otn[NCH - 1][:, 0, :], start=True, stop=True)
            interleave(proj_units(NCH - 1))




_CACHE = {}


def _get_nc(S, D, HC):
    key = (S, D, HC)
    if key not in _CACHE:
        nc = bacc.Bacc(None, target_bir_lowering=False)
        build_core(nc, S, D, HC)
        nc.compile()
        _CACHE[key] = nc
    return _CACHE[key]


def make_in_maps(x, rope_cos, rope_sin, W_qkv, W_out, n_cores=8):
    B, S, D = x.shape
    H = 16
    groups = n_cores // B          # head groups per batch
    HC = H // groups               # heads per core
    M = HC * HD
    MT = M // 128
    DT, NCH = D // 128, S // CH
    perm = rope_perm()
    bf16 = ml_dtypes.bfloat16
    cs, sn = rope_tables(np.asarray(rope_cos), np.asarray(rope_sin), S)
    csn = np.stack([cs.reshape(128, NCH * CH), sn.reshape(128, NCH * CH)],
                   axis=1).reshape(128, 2 * NCH * CH)
    in_maps = []
    xtb_cache = {}
    for c in range(n_cores):
        b, g = divmod(c, groups)
        heads = np.arange(g * HC, (g + 1) * HC)
        qcols = np.concatenate([h * HD + perm for h in heads])
        vcols = np.concatenate([2 * D + h * HD + np.arange(HD) for h in heads])
        if b not in xtb_cache:
            xtb_cache[b] = np.ascontiguousarray(
                np.asarray(x[b]).T.reshape(DT, 128, NCH, CH)
                .transpose(2, 1, 0, 3).reshape(NCH, 128, DT * CH)
            ).astype(bf16)

        def wfmt(wcols):
            return np.ascontiguousarray(
                wcols.reshape(DT, 128, M).transpose(1, 0, 2).reshape(128, DT * M)
            ).astype(bf16)

        wo_np = np.ascontiguousarray(
            W_out[g * M:(g + 1) * M, :].reshape(MT, 128, D)
            .transpose(1, 0, 2).reshape(128, MT * D)).astype(bf16)
        in_maps.append({
            "xt": xtb_cache[b],
            "wq": wfmt(W_qkv[:, qcols]),
            "wk": wfmt(W_qkv[:, D + qcols]),
            "wv": wfmt(W_qkv[:, vcols]),
            "wo": wo_np,
            "csn": np.ascontiguousarray(csn).astype(bf16),
        })
    return in_maps


def unshard_out(res, B, S, D, n_cores=8):
    NCH, NT = S // CH, D // 128
    NT2 = NT // 2
    out = np.zeros((B, S, D), np.float32)
    for c in range(n_cores):
        yt = res.results[c]["yt"].astype(np.float32)  # [NCH, 2, 128, NT2*CH]
        ytf = (yt.reshape(NCH, 2, 128, NT2, CH)
               .transpose(1, 3, 2, 0, 4).reshape(D, S))
        out[c // (n_cores // B)] += ytf.T
    return out


def kernel(x, rope_cos, rope_sin, W_qkv, W_out):
    x = np.asarray(x)
    W_qkv = np.asarray(W_qkv)
    W_out = np.asarray(W_out)
    B, S, D = x.shape
    n_cores = 8
    HC = 16 // (n_cores // B)
    in_maps = make_in_maps(x, rope_cos, rope_sin, W_qkv, W_out, n_cores)
    nc = _get_nc(S, D, HC)
    res = run_bass_kernel_spmd(nc, in_maps, list(range(n_cores)))
    return unshard_out(res, B, S, D, n_cores)



# revision 53
# speedup vs baseline: 1.2288x; 1.0048x over previous
"""Causal self-attention (B=2, S=2048, D=1024, H=16) on 8 TRN2 NeuronCores.

Sharding: core c -> batch b = c//4, head group g = c%4 (4 heads each).
Each core computes the qkv projection for its heads, RoPE, causal flash
attention, and a partial out-projection (row-parallel); the host sums the
4 partials per batch.

Layout strategy (everything "transposed", seq on the free axis):
  xt  = x[b]^T                  [D, S]   bf16 (host-prepped)
  Qt/Kt[m, s] per head          computed as  W[:,m]^T @ xt  (lhsT = W slice)
  V natural [s, m]              computed as  xt_tile^T @ Wv
  St[k, q]  = Kt_tile^T @ Qt    -> exp -> causal mask -> Pt (bf16)
  Ot'[m+1, q] = [V|1]^T @ Pt    (row m==HD is the softmax denominator l[q])
  y^T[n, s] = Wo[:,n]^T @ (Ot/l)  accumulated over m tiles; host sums cores.

RoPE: interleaved rotate made partition-aligned by permuting W columns on
the host; on-device combine = 2 muls + stream_shuffle + add (Vector).

Perf structure (v3, 168us vs 185us baseline):
 - All inputs land via 9 large DMAs on the sync HWDGE ring instead of ~70
   small ones (each dma_start costs ~615ns of serialized issue), so the
   first matmul starts ~3us in and the HAM clock-gate (which halves the PE
   clock after ~3.4us of low activity) never re-throttles mid-kernel.
 - 10 dummy matmuls at t=0 warm the PE clock while the DMAs stream; 8 more
   before the last out-projection cover the exp-bound tail.
 - Work is emitted chunk-by-chunk with attention of chunk ci interleaved
   with the qkv of chunk ci+2 and the out-projection of ci-1 (PE filler
   for the ACT-exp-paced softmax, ~1us per key-tile pair). Emission order
   per engine follows dataflow order pair-by-pair: reordering across pairs
   creates multi-us cross-FIFO convoys (engines execute in FIFO order).
 - Output is evicted to bf16 and written with 2 large DMAs per chunk.
 - RoPE add on GpSimd; l broadcast on GpSimd; evictions on Vector (last
   chunk on Scalar, which is idle once exp finishes).
"""

from contextlib import ExitStack

import numpy as np
import ml_dtypes

import concourse.bass as bass
import concourse.tile as tile
import concourse.mybir as mybir
from concourse import bacc
from concourse.bass_utils import run_bass_kernel_spmd

HD = 64          # head dim
CH = 512         # seq chunk (one PSUM bank of fp32)
_SHUF = [(i + 16) % 32 for i in range(32)]  # swap 16-halves in each quadrant


def rope_perm():
    """Within-head output-column permutation: local row r <- reference col."""
    perm = np.zeros(HD, dtype=np.int64)
    for r in range(HD):
        q, pos = divmod(r, 32)
        x2 = pos >= 16
        f = q * 16 + (pos % 16)
        perm[r] = 2 * f + (1 if x2 else 0)
    return perm


def rope_tables(rope_cos, rope_sin, S):
    """cos/sin tables [128, S] fp32 aligned with the permuted Qt/Kt rows."""
    cs = np.zeros((128, S), np.float32)
    sn = np.zeros((128, S), np.float32)
    for r in range(128):
        rr = r % HD
        q, pos = divmod(rr, 32)
        x2 = pos >= 16
        f = q * 16 + (pos % 16)
        cs[r] = rope_cos[:S, f]
        sn[r] = rope_sin[:S, f] * (-1.0 if x2 else 1.0)
    return cs, sn


def merge_prop(a, b):
    """Merge two unit lists, advancing each proportionally to its length."""
    out = []
    ia = ib = 0
    while ia < len(a) or ib < len(b):
        fa = ia / len(a) if a else 1.0
        fb = ib / len(b) if b else 1.0
        if ia < len(a) and (ib >= len(b) or fa <= fb):
            out.append(a[ia]); ia += 1
        else:
            out.append(b[ib]); ib += 1
    return out


def build_core(nc, S, D, HC):
    """Emit the per-core kernel IR. HC = heads on this core."""
    DT = D // 128           # contraction tiles over model dim
    M = HC * HD             # local qkv width
    MT = M // 128           # m tiles
    NCH = S // CH           # seq chunks
    KPC = CH // 128         # key tiles per chunk
    NT = D // 128           # out-proj n tiles
    NT2 = NT // 2
    HP = HC // 2            # head pairs
    fp32, bf16 = mybir.dt.float32, mybir.dt.bfloat16
    SCALE = float(HD) ** -0.5

    xt_d = nc.declare_dram_parameter("xt", [NCH, 128, DT * CH], bf16, isOutput=False)
    wq_d = nc.declare_dram_parameter("wq", [128, DT * M], bf16, isOutput=False)
    wk_d = nc.declare_dram_parameter("wk", [128, DT * M], bf16, isOutput=False)
    wv_d = nc.declare_dram_parameter("wv", [128, DT * M], bf16, isOutput=False)
    wo_d = nc.declare_dram_parameter("wo", [128, MT * D], bf16, isOutput=False)
    csn_d = nc.declare_dram_parameter("csn", [128, 2 * NCH * CH], bf16, isOutput=False)
    yt_d = nc.declare_dram_parameter("yt", [NCH, 2, 128, NT2 * CH], bf16, isOutput=True)

    with tile.TileContext(nc) as tc, ExitStack() as ctx:
        persist = ctx.enter_context(tc.tile_pool(name="persist", bufs=1))
        mm_ps = ctx.enter_context(tc.tile_pool(name="mm_ps", bufs=2, space="PSUM"))
        st_ps = ctx.enter_context(tc.tile_pool(name="st_ps", bufs=2, space="PSUM"))
        ot_ps = ctx.enter_context(tc.tile_pool(name="ot_ps", bufs=2, space="PSUM"))
        work = ctx.enter_context(tc.tile_pool(name="work", bufs=3))
        pt_pool = ctx.enter_context(tc.tile_pool(name="ptp", bufs=20))
        out_pool = ctx.enter_context(tc.tile_pool(name="outp", bufs=2))

        # ---- persistent tiles -------------------------------------------
        xt = [persist.tile([128, DT, CH], bf16, name=f"xt_{c}") for c in range(NCH)]
        wq = persist.tile([128, DT, M], bf16)
        wk = persist.tile([128, DT, M], bf16)
        wv = persist.tile([128, DT, M], bf16)
        wo = persist.tile([128, MT, D], bf16)
        csn = persist.tile([128, 2, NCH, CH], bf16)
        qt = [persist.tile([128, MT, CH], bf16, name=f"qt_{c}") for c in range(NCH)]
        kt = [persist.tile([128, MT, CH], bf16, name=f"kt_{c}") for c in range(NCH)]
        vsb = [persist.tile([128, KPC, HC, HD + 1], bf16, name=f"vsb_{c}")
               for c in range(NCH)]
        otn = [persist.tile([128, MT, CH], bf16, name=f"otn_{c}") for c in range(NCH)]
        cmask = persist.tile([128, 2, 128], bf16)

        # ---- input DMAs: few and large, on the sync HWDGE ring.  The
        # chunk-0 slice of the rope tables rides ahead of the full table so
        # the first rope (chunk 0 -> the first exp) isn't gated on 1MB.
        csn_dv = csn_d.rearrange("p (i c s) -> p i c s", i=2, c=NCH)
        nc.sync.dma_start(out=wq.rearrange("p t m -> p (t m)"), in_=wq_d[:, :])
        nc.sync.dma_start(out=csn[:, :, 0, :], in_=csn_dv[:, :, 0, :])
        nc.sync.dma_start(
            out=xt[0].rearrange("p t s -> p (t s)"), in_=xt_d[0])
        nc.sync.dma_start(out=csn[:, :, 1:, :], in_=csn_dv[:, :, 1:, :])
        nc.sync.dma_start(out=wk.rearrange("p t m -> p (t m)"), in_=wk_d[:, :])
        nc.sync.dma_start(out=wv.rearrange("p t m -> p (t m)"), in_=wv_d[:, :])
        for c in range(1, NCH):
            nc.sync.dma_start(
                out=xt[c].rearrange("p t s -> p (t s)"), in_=xt_d[c])
        nc.sync.dma_start(out=wo.rearrange("p t n -> p (t n)"), in_=wo_d[:, :])

        # ---- PE warmup: HAM clock un-throttle while DMAs stream ---------
        # ~128 small matmuls keep the PE busy (and at full clock) until the
        # first real matmul's inputs land (~16us); a dummy exp preloads the
        # ACT table set (~2.7us) off the first real exp's critical path.
        warm_w = persist.tile([128, 128], bf16)
        warm_x = persist.tile([128, CH], bf16)
        nc.vector.memset(warm_w[:], 0.0)
        nc.vector.memset(warm_x[:], 0.0)
        warm_act = work.tile([128, 16], fp32, tag="wact")
        nc.scalar.activation(out=warm_act[:], in_=warm_w[:, 0:16],
                             func=mybir.ActivationFunctionType.Exp)
        warm_ps = mm_ps.tile([128, CH], fp32, tag="mm", name="mmps")
        for _ in range(128):
            nc.tensor.matmul(warm_ps[:, 0:128], warm_w[:], warm_w[:],
                             start=True, stop=True)

        # causal mask for the 128-col diagonal window (both head slots):
        # keep j - p >= 0 (query-local j, key-local p) - identical for
        # every diagonal tile; columns past the window are never masked.
        # Built on GpSimd while it is otherwise idle.
        nc.gpsimd.memset(cmask[:], 1.0)
        for i in range(2):
            nc.gpsimd.affine_select(
                out=cmask[:, i, :], in_=cmask[:, i, :],
                compare_op=mybir.AluOpType.is_ge, fill=0.0,
                base=0, pattern=[[1, 128]], channel_multiplier=-1,
            )
        for c in range(NCH):
            nc.vector.memset(vsb[c][:, :, :, HD:HD + 1], 1.0)

        # ---- emission units ---------------------------------------------
        def qk_part(ci, wt, dst):
            for mt in range(MT):
                ps = mm_ps.tile([128, CH], fp32, tag="mm", name="mmps")
                for dt in range(DT):
                    nc.tensor.matmul(
                        ps[:],
                        wt[:, dt, mt * 128:(mt + 1) * 128],
                        xt[ci][:, dt, :],
                        start=(dt == 0), stop=(dt == DT - 1),
                    )
                p1 = work.tile([128, CH], fp32, tag="p1")
                p2 = work.tile([128, CH], fp32, tag="p2")
                p2s = work.tile([128, CH], fp32, tag="p2s")
                nc.vector.tensor_mul(p1[:], ps[:], csn[:, 0, ci, :])
                nc.vector.tensor_mul(p2[:], ps[:], csn[:, 1, ci, :])
                nc.vector.stream_shuffle(p2s[:], p2[:], mask=_SHUF)
                nc.vector.tensor_add(dst[ci][:, mt, :], p1[:], p2s[:])

        def v_part(ci):
            for sl in range(KPC):
                ps = mm_ps.tile([128, M], fp32, tag="mm", name="vps")
                for dt in range(DT):
                    nc.tensor.matmul(
                        ps[:],
                        xt[ci][:, dt, sl * 128:(sl + 1) * 128],
                        wv[:, dt, :],
                        start=(dt == 0), stop=(dt == DT - 1),
                    )
                nc.vector.tensor_copy(
                    vsb[ci][:, sl, :, 0:HD],
                    ps.rearrange("p (h d) -> p h d", h=HC),
                )

        def attn_part(ci, hp):
            nkt = (ci + 1) * KPC
            heads = (2 * hp, 2 * hp + 1)
            mt = hp
            ots = {}
            for h in heads:
                ots[h] = ot_ps.tile([128, CH], fp32, tag="ot", name=f"ot_{h}")
            pts = []
            for kj in range(nkt):
                tidx = kj - ci * KPC
                trim = max(0, tidx) * 128
                kc, kl = divmod(kj, KPC)
                stp = st_ps.tile([128, 2, CH], fp32, tag="st")
                for i, h in enumerate(heads):
                    base = (h % 2) * 64
                    nc.tensor.matmul(
                        stp[:, i, trim:],
                        kt[kc][base:base + HD, mt, kl * 128:(kl + 1) * 128],
                        qt[ci][base:base + HD, mt, trim:],
                        start=True, stop=True,
                    )
                pt = pt_pool.tile([128, 2, CH], bf16, tag="pt")
                nc.scalar.activation(
                    out=pt[:, :, trim:], in_=stp[:, :, trim:],
                    func=mybir.ActivationFunctionType.Exp, scale=SCALE,
                )
                if tidx >= 0:
                    nc.vector.tensor_mul(
                        pt[:, :, trim:trim + 128], pt[:, :, trim:trim + 128],
                        cmask[:, :, :],
                    )
                pts.append((pt, trim))
            for i, h in enumerate(heads):
                for kj in range(nkt):
                    pt, trim = pts[kj]
                    kc, kl = divmod(kj, KPC)
                    nc.tensor.matmul(
                        ots[h][0:HD + 1, trim:],
                        vsb[kc][:, kl, h, :],
                        pt[:, i, trim:],
                        start=(kj == 0), stop=(kj == nkt - 1),
                    )
            for h in heads:
                base = (h % 2) * 64
                ot = ots[h]
                l_sb = work.tile([1, CH], fp32, tag="l")
                nc.vector.tensor_copy(l_sb[:], ot[HD:HD + 1, :])
                rl = work.tile([1, CH], fp32, tag="rl")
                nc.vector.reciprocal_approx_fast(rl[:], l_sb[:])
                lb = work.tile([64, CH], fp32, tag="lb")
                nc.gpsimd.partition_broadcast(lb[:], rl[0:1, :])
                nc.vector.tensor_mul(
                    otn[ci][base:base + HD, mt, :], ot[0:HD, :], lb[:],
                )

        yts = {}

        def proj_part(ci, half):
            if half == 0:
                yts[ci] = out_pool.tile([128, NT, CH], bf16, tag="yt", name="yt")
            for nt in range(half * NT2, (half + 1) * NT2):
                ps = mm_ps.tile([128, CH], fp32, tag="mm", name="mmps")
                for mt2 in range(MT):
                    nc.tensor.matmul(
                        ps[:],
                        wo[:, mt2, nt * 128:(nt + 1) * 128],
                        otn[ci][:, mt2, :],
                        start=(mt2 == 0), stop=(mt2 == MT - 1),
                    )
                # last chunk: alternate Scalar/Vector (both idle once the
                # exps finish) so the tail eviction chain halves.
                if ci == NCH - 1 and nt % 2 == 0:
                    nc.scalar.copy(yts[ci][:, nt, :], ps[:])
                else:
                    nc.vector.tensor_copy(yts[ci][:, nt, :], ps[:])
            # last chunk's second half goes out on the scalar HWDGE ring
            # (idle by then) so the two tail writebacks transfer in
            # parallel instead of back-to-back on sync.
            eng = nc.scalar if (ci == NCH - 1 and half == 1) else nc.sync
            eng.dma_start(
                out=yt_d[ci, half],
                in_=yts[ci][:, half * NT2:(half + 1) * NT2, :]
                .rearrange("p t s -> p (t s)"),
            )

        from itertools import zip_longest

        def interleave(*streams):
            for group in zip_longest(*streams):
                for fn in group:
                    if fn is not None:
                        fn()

        def qkv_units(ci):
            return [lambda: qk_part(ci, wq, qt),
                    lambda: qk_part(ci, wk, kt),
                    lambda: v_part(ci)]

        def attn_units(ci):
            return [(lambda hp=hp: attn_part(ci, hp)) for hp in range(HP)]

        def proj_units(ci):
            return [lambda: proj_part(ci, 0), lambda: proj_part(ci, 1)]

        interleave(qkv_units(0))
        if NCH == 1:
            interleave(attn_units(0))
            interleave(proj_units(0))
        else:
            interleave(qkv_units(1))
            for ci in range(NCH - 2):
                streams = [attn_units(ci), qkv_units(ci + 2)]
                if ci >= 1:
                    streams.append(proj_units(ci - 1))
                interleave(*streams)
            interleave(attn_units(NCH - 2),
                       proj_units(NCH - 3) if NCH >= 3 else [])
            interleave(attn_units(NCH - 1),
                       proj_units(NCH - 2) if NCH >= 2 else [])
            ka_ps = mm_ps.tile([128, CH], fp32, tag="mm", name="ka_ps")
            for _ in range(8):
                nc.tensor.matmul(ka_ps[:], warm_w[:],
                                 otn[NCH - 1][:, 0, :], start=True, stop=True)
            interleave(proj_units(NCH - 1))




_CACHE = {}


def _get_nc(S, D, HC):
    key = (S, D, HC)
    if key not in _CACHE:
        nc = bacc.Bacc(None, target_bir_lowering=False)
        build_core(nc, S, D, HC)
        nc.compile()
        _CACHE[key] = nc
    return _CACHE[key]


def make_in_maps(x, rope_cos, rope_sin, W_qkv, W_out, n_cores=8):
    B, S, D = x.shape
    H = 16
    groups = n_cores // B          # head groups per batch
    HC = H // groups               # heads per core
    M = HC * HD
    MT = M // 128
    DT, NCH = D // 128, S // CH
    perm = rope_perm()
    bf16 = ml_dtypes.bfloat16
    cs, sn = rope_tables(np.asarray(rope_cos), np.asarray(rope_sin), S)
    csn = np.stack([cs.reshape(128, NCH * CH), sn.reshape(128, NCH * CH)],
                   axis=1).reshape(128, 2 * NCH * CH)
    in_maps = []
    xtb_cache = {}
    for c in range(n_cores):
        b, g = divmod(c, groups)
        heads = np.arange(g * HC, (g + 1) * HC)
        qcols = np.concatenate([h * HD + perm for h in heads])
        vcols = np.concatenate([2 * D + h * HD + np.arange(HD) for h in heads])
        if b not in xtb_cache:
            xtb_cache[b] = np.ascontiguousarray(
                np.asarray(x[b]).T.reshape(DT, 128, NCH, CH)
                .transpose(2, 1, 0, 3).reshape(NCH, 128, DT * CH)
            ).astype(bf16)

        def wfmt(wcols):
            return np.ascontiguousarray(
                wcols.reshape(DT, 128, M).transpose(1, 0, 2).reshape(128, DT * M)
            ).astype(bf16)

        wo_np = np.ascontiguousarray(
            W_out[g * M:(g + 1) * M, :].reshape(MT, 128, D)
            .transpose(1, 0, 2).reshape(128, MT * D)).astype(bf16)
        in_maps.append({
            "xt": xtb_cache[b],
            "wq": wfmt(W_qkv[:, qcols]),
            "wk": wfmt(W_qkv[:, D + qcols]),
            "wv": wfmt(W_qkv[:, vcols]),
            "wo": wo_np,
            "csn": np.ascontiguousarray(csn).astype(bf16),
        })
    return in_maps


def unshard_out(res, B, S, D, n_cores=8):
    NCH, NT = S // CH, D // 128
    NT2 = NT // 2
    out = np.zeros((B, S, D), np.float32)
    for c in range(n_cores):
        yt = res.results[c]["yt"].astype(np.float32)  # [NCH, 2, 128, NT2*CH]
        ytf = (yt.reshape(NCH, 2, 128, NT2, CH)
               .transpose(1, 3, 2, 0, 4).reshape(D, S))
        out[c // (n_cores // B)] += ytf.T
    return out


def kernel(x, rope_cos, rope_sin, W_qkv, W_out):
    x = np.asarray(x)
    W_qkv = np.asarray(W_qkv)
    W_out = np.asarray(W_out)
    B, S, D = x.shape
    n_cores = 8
    HC = 16 // (n_cores // B)
    in_maps = make_in_maps(x, rope_cos, rope_sin, W_qkv, W_out, n_cores)
    nc = _get_nc(S, D, HC)
    res = run_bass_kernel_spmd(nc, in_maps, list(range(n_cores)))
    return unshard_out(res, B, S, D, n_cores)

